# revision 10
# baseline (speedup 1.0000x reference)
"""Trainium2 Bass kernel for AttentionAggregate (GAT-style neighbor aggregation).

Reference computation (per node n, neighbors k=0..K-1):
    pt = target @ W.T + b                      # [N, D]
    pm = middle @ W.T + b                      # [N, K, D]
    score = leaky_relu((pt[:,None,:] + pm) @ a_w.T + a_b)
    coef  = softmax(score, axis=K)
    out   = sum_k coef * middle                # [N, D]

Key algebraic simplification: the W-projection only enters through the dot
with a_w, so with u = a_w @ W (a single D-vector) and c = 2*(a_w.b) + a_b:
    score[n,k] = target[n].u + middle[n,k].u + c
This removes all large matmuls; the kernel is a memory-bound pass over
`middle` (512 MiB) with per-node softmax weighting.

Sharding: data-parallel over nodes. N=16384 nodes split across 8 cores
(2048 nodes each); W/b/a_w/a_b replicated; no cross-core communication.

Engine assignment per 128-node tile [128, K, D] (node on partition),
designed so each engine's work fits under the ~12.7 us/tile DMA slot
(4 MiB tile at ~330 GB/s):
  DVE:  32x fused tensor_tensor_reduce (mid[:,k,:]*u, accum -> s[:,k]),
        + bias add, leaky (scalar_tensor_tensor), reciprocal  (~10 us)
  ACT:  exp with fused denominator accum, PSUM evacuation with 1/den
        per-partition scale, out/setup DMAs on the 2nd HWDGE queue (~1.5 us)
  Pool: 32x diag(e_k) generation (tensor_scalar_mul on identity) (~10 us)
  PE:   32x accumulating matmuls diag(e_k) @ mid_bf16[:,k,:] (bf16 =
        1 cycle/row, cheap LDWEIGHTS; ACT makes the bf16 copy)  (~6 us)
  Sync: middle-tile DMAs only, so the main HWDGE queue streams
        back-to-back.

Softmax is computed without max-subtraction: scores are O(+-8) here
(u ~ unit norm, inputs ~ N(0,1)), so exp stays well inside f32 range.
"""

from contextlib import ExitStack

import numpy as np

import concourse.bass as bass
import concourse.tile as tile
from concourse import mybir
from concourse.bass_utils import run_bass_kernel_spmd

N_CORES = 8
N, K, D = 16384, 32, 256
NS = N // N_CORES  # nodes per core
P = 128
F32 = mybir.dt.float32
BF16 = mybir.dt.bfloat16
ALU = mybir.AluOpType
AF = mybir.ActivationFunctionType
AX = mybir.AxisListType
NEG_SLOPE = 0.01


def emit_kernel(tc, out, tgt, mid, W, b, a_w, a_b, ident, ns):
    nc = tc.nc
    nt = ns // P  # node tiles per core
    with ExitStack() as ctx:
        singles = ctx.enter_context(tc.tile_pool(name="singles", bufs=1))
        mids = ctx.enter_context(tc.tile_pool(name="mids", bufs=3))
        midbs = ctx.enter_context(tc.tile_pool(name="midbs", bufs=2))
        small = ctx.enter_context(tc.tile_pool(name="small", bufs=4))
        scr = ctx.enter_context(tc.tile_pool(name="scr", bufs=1))
        dgs = ctx.enter_context(tc.tile_pool(name="dgs", bufs=6))
        psum = ctx.enter_context(tc.tile_pool(name="psum", bufs=2, space="PSUM"))
        opsum = ctx.enter_context(tc.tile_pool(name="opsum", bufs=2, space="PSUM"))
        outs = ctx.enter_context(tc.tile_pool(name="outs", bufs=3))

        # ---- setup: u = a_w @ W, c = 2*(a_w.b) + a_b ----
        # All setup DMAs go on the Activation HWDGE queue; the Sync queue is
        # reserved for the big middle-tile streams.
        W0 = singles.tile([P, D], F32)
        W1 = singles.tile([P, D], F32)
        nc.scalar.dma_start(W0, W[0:P, :])
        nc.scalar.dma_start(W1, W[P : 2 * P, :])
        # a_w transposed onto partitions: awT[p, g] = a_w[0, g*128 + p]
        awT = singles.tile([P, 2], F32)
        nc.scalar.dma_start(awT, a_w.rearrange("o (g p) -> p (g o)", g=2))
        b_row = singles.tile([1, D], F32)
        nc.scalar.dma_start(b_row, b.unsqueeze(0))
        aw_row = singles.tile([1, D], F32)
        nc.scalar.dma_start(aw_row, a_w)
        ab_t = singles.tile([1, 1], F32)
        nc.scalar.dma_start(ab_t, a_b.unsqueeze(0))
        id_t = singles.tile([P, P], F32)
        nc.scalar.dma_start(id_t, ident)
        # target, all tiles at once: tg_all[p, t, d] = tgt[t*128+p, d]
        tg_all = singles.tile([P, nt, D], F32)
        nc.scalar.dma_start(tg_all, tgt.rearrange("(t p) d -> p t d", p=P))

        # Wsc[d, e] = a_w[d] * W[d, e]
        Wsc0 = singles.tile([P, D], F32)
        Wsc1 = singles.tile([P, D], F32)
        nc.vector.tensor_scalar_mul(Wsc0, W0, awT[:, 0:1])
        nc.vector.tensor_scalar_mul(Wsc1, W1, awT[:, 1:2])
        ones_col = singles.tile([P, 1], F32)
        ones_row = singles.tile([1, P], F32)
        nc.vector.memset(ones_col, 1.0)
        nc.vector.memset(ones_row, 1.0)
        # u[e] = sum_d Wsc[d, e]  (partition reduction via PE)
        u_ps = psum.tile([1, D], F32)
        nc.tensor.matmul(u_ps, ones_col, Wsc0, start=True, stop=False)
        nc.tensor.matmul(u_ps, ones_col, Wsc1, start=False, stop=True)
        u_row = singles.tile([1, D], F32)
        nc.scalar.copy(u_row, u_ps)

        # c = 2*(b . a_w) + a_b   (fused mul+reduce)
        baw_scr = scr.tile([1, D], F32, tag="baw_scr")
        baw = singles.tile([1, 1], F32)
        nc.vector.scalar_tensor_tensor(
            out=baw_scr, in0=b_row, scalar=0.0, in1=aw_row,
            op0=ALU.bypass, op1=ALU.mult, accum_out=baw,
        )
        c_s = singles.tile([1, 1], F32)
        nc.scalar.activation(c_s, baw, AF.Identity, bias=ab_t, scale=2.0)

        # broadcast u, c across all 128 partitions via PE outer product
        ub_ps = psum.tile([P, D], F32)
        nc.tensor.matmul(ub_ps, ones_row, u_row, start=True, stop=True)
        u_b = singles.tile([P, D], F32)
        nc.scalar.copy(u_b, ub_ps)
        cb_ps = psum.tile([P, 1], F32)
        nc.tensor.matmul(cb_ps, ones_row, c_s, start=True, stop=True)
        c_b = singles.tile([P, 1], F32)
        nc.scalar.copy(c_b, cb_ps)

        # per-node constant: stc_c[:, t] = target[t].u  (c added with sb)
        stc_c = singles.tile([P, nt], F32)
        stc_scr = scr.tile([P, D], F32, tag="stc_scr")
        ph1_scr = scr.tile([P, D], F32, tag="ph1_scr")

        # ---- main loop over node tiles ----
        for t in range(nt):
            m = mids.tile([P, K, D], F32, tag="mid")
            nc.sync.dma_start(m, mid[t * P : (t + 1) * P, :, :])

            # bf16 copy for the PE aggregation (ACT is otherwise idle)
            mb = midbs.tile([P, K, D], BF16, tag="midb")
            nc.scalar.copy(mb, m)

            nc.vector.scalar_tensor_tensor(
                out=stc_scr, in0=tg_all[:, t, :], scalar=0.0, in1=u_b,
                op0=ALU.bypass, op1=ALU.mult,
                accum_out=stc_c[:, t : t + 1],
            )

            # phase 1: s[:, k] = sum_d mid[:, k, :] * u  (fused mul+reduce)
            s = small.tile([P, K], F32, tag="s")
            for k in range(K):
                nc.vector.scalar_tensor_tensor(
                    out=ph1_scr, in0=m[:, k, :], scalar=0.0, in1=u_b,
                    op0=ALU.bypass, op1=ALU.mult,
                    accum_out=s[:, k : k + 1],
                )

            # scores: sb = s + target.u + c; leaky = max(sb, 0.01*sb)
            sb = small.tile([P, K], F32, tag="sb")
            nc.vector.tensor_scalar(
                out=sb, in0=s, scalar1=stc_c[:, t : t + 1], scalar2=c_b,
                op0=ALU.add, op1=ALU.add,
            )
            s2 = small.tile([P, K], F32, tag="s2")
            nc.vector.scalar_tensor_tensor(
                out=s2, in0=sb, scalar=NEG_SLOPE, in1=sb,
                op0=ALU.mult, op1=ALU.max,
            )

            # softmax over k without max-subtraction; denominator fused
            e = small.tile([P, K], F32, tag="e")
            den = small.tile([P, 1], F32, tag="den")
            nc.scalar.activation(e, s2, AF.Exp, accum_out=den)
            rcp = small.tile([P, 1], F32, tag="rcp")
            nc.vector.reciprocal(rcp, den)

            # phase 2 on PE: o_ps = sum_k diag(e[:,k]) @ mid_bf16[:,k,:]
            # (unnormalized; 1/den applied at PSUM evacuation). bf16 runs the
            # PE at 1 cycle/row and makes LDWEIGHTS 4x cheaper than f32.
            o_ps = opsum.tile([P, D], F32, tag="o_ps")
            for k in range(K):
                dg = dgs.tile([P, P], BF16, tag="dg")
                nc.gpsimd.tensor_scalar_mul(dg, id_t, e[:, k : k + 1])
                nc.tensor.matmul(
                    o_ps, dg, mb[:, k, :],
                    start=(k == 0), stop=(k == K - 1), skip_group_check=True,
                )
            o_sb = outs.tile([P, D], F32, tag="o_sb")
            nc.scalar.mul(o_sb, o_ps, rcp[:, 0:1])
            nc.scalar.dma_start(out[t * P : (t + 1) * P, :], o_sb)


def build_nc(ns=NS):
    nc = bass.Bass("TRN2", debug=False, num_devices=N_CORES)
    tgt = nc.dram_tensor("target", [ns, D], F32, kind="ExternalInput").ap()
    mid = nc.dram_tensor("middle", [ns, K, D], F32, kind="ExternalInput").ap()
    W = nc.dram_tensor("W", [D, D], F32, kind="ExternalInput").ap()
    b = nc.dram_tensor("b", [D], F32, kind="ExternalInput").ap()
    a_w = nc.dram_tensor("a_w", [1, D], F32, kind="ExternalInput").ap()
    a_b = nc.dram_tensor("a_b", [1], F32, kind="ExternalInput").ap()
    ident = nc.dram_tensor("ident", [P, P], F32, kind="ExternalInput").ap()
    out = nc.dram_tensor("out", [ns, D], F32, kind="ExternalOutput").ap()
    with tile.TileContext(nc) as tc:
        emit_kernel(tc, out, tgt, mid, W, b, a_w, a_b, ident, ns)
    import bass_rust as _br

    # Split multi-wait instructions (walrus allows at most 1 sync wait per
    # instruction; Tile can emit more after multi-DMA dependencies).
    _br.generate_event_semaphores(nc)
    return nc


_NC_CACHE = {}


def _get_nc(ns=NS):
    if ns not in _NC_CACHE:
        _NC_CACHE[ns] = build_nc(ns)
    return _NC_CACHE[ns]


def make_in_maps(target, middle, W, b, a_w, a_b):
    target = np.ascontiguousarray(np.asarray(target, dtype=np.float32))
    middle = np.ascontiguousarray(np.asarray(middle, dtype=np.float32))
    W = np.ascontiguousarray(np.asarray(W, dtype=np.float32))
    b = np.ascontiguousarray(np.asarray(b, dtype=np.float32))
    a_w = np.ascontiguousarray(np.asarray(a_w, dtype=np.float32))
    a_b = np.ascontiguousarray(np.asarray(a_b, dtype=np.float32))
    ident = np.eye(P, dtype=np.float32)
    tgt_shards = np.split(target, N_CORES, axis=0)
    mid_shards = np.split(middle, N_CORES, axis=0)
    return [
        {
            "target": tgt_shards[i],
            "middle": mid_shards[i],
            "W": W,
            "b": b,
            "a_w": a_w,
            "a_b": a_b,
            "ident": ident,
        }
        for i in range(N_CORES)
    ]


def run_sharded(in_maps, **kwargs):
    nc = _get_nc(in_maps[0]["target"].shape[0])
    res = run_bass_kernel_spmd(nc, in_maps, list(range(N_CORES)), **kwargs)
    full = np.concatenate([r["out"] for r in res.results], axis=0)
    return full, res


def kernel(target, middle, W, b, a_w, a_b):
    in_maps = make_in_maps(target, middle, W, b, a_w, a_b)
    full, _ = run_sharded(in_maps)
    return full


# revision 11
# speedup vs baseline: 3.9020x; 3.9020x over previous
"""Trainium2 Bass kernel for AttentionAggregate (GAT-style neighbor aggregation).

Reference computation (per node n, neighbors k=0..K-1):
    pt = target @ W.T + b                      # [N, D]
    pm = middle @ W.T + b                      # [N, K, D]
    score = leaky_relu((pt[:,None,:] + pm) @ a_w.T + a_b)
    coef  = softmax(score, axis=K)
    out   = sum_k coef * middle                # [N, D]

Key algebraic simplification: the W-projection only enters through the dot
with a_w, so with u = a_w @ W (a single D-vector) and c = 2*(a_w.b) + a_b:
    score[n,k] = target[n].u + middle[n,k].u + c
This removes all large matmuls; the kernel is a memory-bound pass over
`middle` (512 MiB) with per-node softmax weighting.

Sharding: data-parallel over nodes. N=16384 nodes split across 8 cores
(2048 nodes each); W/b/a_w/a_b replicated; no cross-core communication.

Engine assignment per 128-node tile (node on partition), sized so every
engine fits under the ~12.7 us/tile DMA slot (4 MiB HBM read at ~330 GB/s):
  Pool: issues the middle-tile DMAs as casting SWDGE transfers — DMA
        converts f32 (HBM) -> fp16 (SBUF) in flight, so no on-chip convert
        pass is needed. fp16 (10 mantissa bits) keeps rounding ~4x below
        bf16; all values here are O(+-10), well inside fp16 range.
  DVE:  one big fp16 multiply m2 = mid*u (2x perf mode for 16-bit) + one
        segmented fp16 reduce -> s[n,k], plus bias/leaky/reciprocal smalls.
  ACT:  exp with fused denominator accumulation, 32x diag(e_k) generation
        (Copy activation with per-partition scale), PSUM evacuation scaled
        by 1/den.
  PE:   32x accumulating fp16 matmuls diag(e_k) @ mid[:,k,:] (1 cycle/row).
  Sync: setup + output DMAs on the SP HWDGE queue.

Softmax is computed without max-subtraction: scores are O(+-8) here, exp
stays well inside f32 range.
"""

from contextlib import ExitStack

import numpy as np

import concourse.bass as bass
import concourse.tile as tile
from concourse import mybir
from concourse.bass_utils import run_bass_kernel_spmd

N_CORES = 8
N, K, D = 16384, 32, 256
NS = N // N_CORES  # nodes per core
P = 128
F32 = mybir.dt.float32
F16 = mybir.dt.float16
ALU = mybir.AluOpType
AF = mybir.ActivationFunctionType
AX = mybir.AxisListType
NEG_SLOPE = 0.01


def emit_kernel(tc, out, tgt, mid, W, b, a_w, a_b, ident, ns):
    nc = tc.nc
    nt = ns // P  # node tiles per core
    with ExitStack() as ctx:
        singles = ctx.enter_context(tc.tile_pool(name="singles", bufs=1))
        mids = ctx.enter_context(tc.tile_pool(name="mids", bufs=4))
        small = ctx.enter_context(tc.tile_pool(name="small", bufs=4))
        scr = ctx.enter_context(tc.tile_pool(name="scr", bufs=1))
        dgs = ctx.enter_context(tc.tile_pool(name="dgs", bufs=8))
        psum = ctx.enter_context(tc.tile_pool(name="psum", bufs=2, space="PSUM"))
        opsum = ctx.enter_context(tc.tile_pool(name="opsum", bufs=2, space="PSUM"))
        outs = ctx.enter_context(tc.tile_pool(name="outs", bufs=3))

        # ---- setup: u = a_w @ W, c = 2*(a_w.b) + a_b ----
        # Setup DMAs ride the SP HWDGE queue; middle streams via Pool SWDGE.
        W0 = singles.tile([P, D], F32)
        W1 = singles.tile([P, D], F32)
        nc.sync.dma_start(W0, W[0:P, :])
        nc.sync.dma_start(W1, W[P : 2 * P, :])
        # a_w transposed onto partitions: awT[p, g] = a_w[0, g*128 + p]
        awT = singles.tile([P, 2], F32)
        nc.sync.dma_start(awT, a_w.rearrange("o (g p) -> p (g o)", g=2))
        b_row = singles.tile([1, D], F32)
        nc.sync.dma_start(b_row, b.unsqueeze(0))
        aw_row = singles.tile([1, D], F32)
        nc.sync.dma_start(aw_row, a_w)
        ab_t = singles.tile([1, 1], F32)
        nc.sync.dma_start(ab_t, a_b.unsqueeze(0))
        id_t = singles.tile([P, P], F32)
        nc.sync.dma_start(id_t, ident)
        # target, all tiles at once: tg_all[p, t, d] = tgt[t*128+p, d]
        tg_all = singles.tile([P, nt, D], F32)
        nc.sync.dma_start(tg_all, tgt.rearrange("(t p) d -> p t d", p=P))

        # Wsc[d, e] = a_w[d] * W[d, e]
        Wsc0 = singles.tile([P, D], F32)
        Wsc1 = singles.tile([P, D], F32)
        nc.vector.tensor_scalar_mul(Wsc0, W0, awT[:, 0:1])
        nc.vector.tensor_scalar_mul(Wsc1, W1, awT[:, 1:2])
        ones_col = singles.tile([P, 1], F32)
        ones_row = singles.tile([1, P], F32)
        nc.vector.memset(ones_col, 1.0)
        nc.vector.memset(ones_row, 1.0)
        # u[e] = sum_d Wsc[d, e]  (partition reduction via PE)
        u_ps = psum.tile([1, D], F32)
        nc.tensor.matmul(u_ps, ones_col, Wsc0, start=True, stop=False)
        nc.tensor.matmul(u_ps, ones_col, Wsc1, start=False, stop=True)
        u_row = singles.tile([1, D], F32)
        nc.scalar.copy(u_row, u_ps)

        # c = 2*(b . a_w) + a_b   (fused mul+reduce)
        baw_scr = scr.tile([1, D], F32, tag="baw_scr")
        baw = singles.tile([1, 1], F32)
        nc.vector.scalar_tensor_tensor(
            out=baw_scr, in0=b_row, scalar=0.0, in1=aw_row,
            op0=ALU.bypass, op1=ALU.mult, accum_out=baw,
        )
        c_s = singles.tile([1, 1], F32)
        nc.scalar.activation(c_s, baw, AF.Identity, bias=ab_t, scale=2.0)

        # broadcast u, c across all 128 partitions via PE outer product
        ub_ps = psum.tile([P, D], F32)
        nc.tensor.matmul(ub_ps, ones_row, u_row, start=True, stop=True)
        u_b = singles.tile([P, D], F32)
        nc.scalar.copy(u_b, ub_ps)
        u_h = singles.tile([P, D], F16)
        nc.scalar.copy(u_h, ub_ps)
        cb_ps = psum.tile([P, 1], F32)
        nc.tensor.matmul(cb_ps, ones_row, c_s, start=True, stop=True)
        c_b = singles.tile([P, 1], F32)
        nc.scalar.copy(c_b, cb_ps)

        # per-node constant: stc[:, t] = target[t].u  (c folded in per tile)
        stc = singles.tile([P, nt], F32)
        tg_scr = scr.tile([P, nt, D], F32, tag="tg_scr")
        nc.vector.tensor_mul(tg_scr, tg_all, u_b.unsqueeze(1).broadcast_to([P, nt, D]))
        nc.vector.reduce_sum(stc, tg_scr, AX.X)

        m2h_scr = scr.tile([P, K, D], F16, tag="m2h_scr")
        u_h_bc = u_h.unsqueeze(1).broadcast_to([P, K, D])

        # ---- main loop over node tiles ----
        for t in range(nt):
            # casting DMA: f32 in HBM -> fp16 in SBUF (Pool SWDGE queue)
            m = mids.tile([P, K, D], F16, tag="mid")
            nc.gpsimd.dma_start(m, mid[t * P : (t + 1) * P, :, :])

            # phase 1: m2 = mid * u (fp16, 2x); s[:, k] = sum_d m2[:, k, :]
            s = small.tile([P, K], F16, tag="s")
            nc.vector.tensor_mul(m2h_scr, m, u_h_bc)
            with nc.allow_low_precision("fp16 scores, tolerance is 2e-2"):
                nc.vector.reduce_sum(s, m2h_scr, AX.X)

            # scores: sb = s + target.u + c; leaky = max(sb, 0.01*sb)
            sb = small.tile([P, K], F16, tag="sb")
            nc.vector.tensor_scalar(
                out=sb, in0=s, scalar1=stc[:, t : t + 1], scalar2=c_b,
                op0=ALU.add, op1=ALU.add,
            )
            s2 = small.tile([P, K], F16, tag="s2")
            nc.vector.scalar_tensor_tensor(
                out=s2, in0=sb, scalar=NEG_SLOPE, in1=sb,
                op0=ALU.mult, op1=ALU.max,
            )

            # softmax over k without max-subtraction; denominator fused
            e = small.tile([P, K], F32, tag="e")
            den = small.tile([P, 1], F32, tag="den")
            nc.scalar.activation(e, s2, AF.Exp, accum_out=den)
            rcp = small.tile([P, 1], F32, tag="rcp")
            nc.vector.reciprocal(rcp, den)

            # phase 2 on PE: o_ps = sum_k diag(e[:,k]) @ mid[:,k,:]  (fp16,
            # 1 cycle/row; unnormalized — 1/den applied at PSUM evacuation).
            # diag(e_k) built on ACT: Copy with per-partition scale e[:,k].
            o_ps = opsum.tile([P, D], F32, tag="o_ps")
            for k in range(K):
                dg = dgs.tile([P, P], F16, tag="dg")
                nc.scalar.mul(dg, id_t, e[:, k : k + 1])
                nc.tensor.matmul(
                    o_ps, dg, m[:, k, :],
                    start=(k == 0), stop=(k == K - 1), skip_group_check=True,
                )
            o_sb = outs.tile([P, D], F32, tag="o_sb")
            nc.scalar.mul(o_sb, o_ps, rcp[:, 0:1])
            nc.sync.dma_start(out[t * P : (t + 1) * P, :], o_sb)


def build_nc(ns=NS):
    nc = bass.Bass("TRN2", debug=False, num_devices=N_CORES)
    tgt = nc.dram_tensor("target", [ns, D], F32, kind="ExternalInput").ap()
    mid = nc.dram_tensor("middle", [ns, K, D], F32, kind="ExternalInput").ap()
    W = nc.dram_tensor("W", [D, D], F32, kind="ExternalInput").ap()
    b = nc.dram_tensor("b", [D], F32, kind="ExternalInput").ap()
    a_w = nc.dram_tensor("a_w", [1, D], F32, kind="ExternalInput").ap()
    a_b = nc.dram_tensor("a_b", [1], F32, kind="ExternalInput").ap()
    ident = nc.dram_tensor("ident", [P, P], F32, kind="ExternalInput").ap()
    out = nc.dram_tensor("out", [ns, D], F32, kind="ExternalOutput").ap()
    with tile.TileContext(nc) as tc:
        emit_kernel(tc, out, tgt, mid, W, b, a_w, a_b, ident, ns)
    import bass_rust as _br

    # Split multi-wait instructions (walrus allows at most 1 sync wait per
    # instruction; Tile can emit more after multi-DMA dependencies).
    _br.generate_event_semaphores(nc)
    return nc


_NC_CACHE = {}


def _get_nc(ns=NS):
    if ns not in _NC_CACHE:
        _NC_CACHE[ns] = build_nc(ns)
    return _NC_CACHE[ns]


def make_in_maps(target, middle, W, b, a_w, a_b):
    target = np.ascontiguousarray(np.asarray(target, dtype=np.float32))
    middle = np.ascontiguousarray(np.asarray(middle, dtype=np.float32))
    W = np.ascontiguousarray(np.asarray(W, dtype=np.float32))
    b = np.ascontiguousarray(np.asarray(b, dtype=np.float32))
    a_w = np.ascontiguousarray(np.asarray(a_w, dtype=np.float32))
    a_b = np.ascontiguousarray(np.asarray(a_b, dtype=np.float32))
    ident = np.eye(P, dtype=np.float32)
    tgt_shards = np.split(target, N_CORES, axis=0)
    mid_shards = np.split(middle, N_CORES, axis=0)
    return [
        {
            "target": tgt_shards[i],
            "middle": mid_shards[i],
            "W": W,
            "b": b,
            "a_w": a_w,
            "a_b": a_b,
            "ident": ident,
        }
        for i in range(N_CORES)
    ]


def run_sharded(in_maps, **kwargs):
    nc = _get_nc(in_maps[0]["target"].shape[0])
    res = run_bass_kernel_spmd(nc, in_maps, list(range(N_CORES)), **kwargs)
    full = np.concatenate([r["out"] for r in res.results], axis=0)
    return full, res


def kernel(target, middle, W, b, a_w, a_b):
    in_maps = make_in_maps(target, middle, W, b, a_w, a_b)
    full, _ = run_sharded(in_maps)
    return full
